# revision 1
# baseline (speedup 1.0000x reference)
"""Trainium2 Bass kernel for nn_ModelNew_3556232921872 (dense_cnn).

Pipeline per sample:
  x_conv = conv3x3(x, W) + b
  acc    = depthwise3x3(x_conv, diag(W)) + b
  group stats over channels per pixel -> norm = (acc - mean_c) * rsqrt(var+eps)
  norm = norm * gamma + beta
  fused = tanh(norm) * clip(norm/6 + 0.5, 0, 1)
  out   = logsumexp(x_conv + fused, channels)          # [1, H, W]

Sharding: data-parallel over batch, B=16 -> 2 samples per NeuronCore x 8.

Implementation notes:
 - conv as 6 matmul passes: 3 passes K=128 (tap pairs (dy,0)+(dy,1) via a
   column-shifted second SBUF copy of the input) + 3 passes K=64 (taps (dy,2)),
   accumulated in PSUM. Depthwise conv: 9 passes K=64 with diagonal lhsT; the
   per-pixel channel-group mean is folded into the center tap
   (diag(wd) - G/8), so the DW psum directly yields t1 = acc - mean_c + bias.
 - Single ACT table set (natural_log_exp_and_others): rsqrt(v)=exp(-0.5 ln v),
   tanh(x)=1-2/(1+exp(2x)) with reciprocal_approx_fast on DVE, final ln on ACT.
"""
import numpy as np

import concourse.bass as bass
import concourse.bacc as bacc
import concourse.mybir as mybir
from concourse.tile import TileContext
from concourse.bass_utils import run_bass_kernel_spmd
from concourse.mybir import AluOpType, ActivationFunctionType

F32 = mybir.dt.float32
BF16 = mybir.dt.bfloat16

# ---- custom fused DVE ops ----
from concourse.dve_spec import (Spec, Src0, Src1, C0, C1, C2, Zero, One,
                                maxx, minn, lower)
from concourse.dve_spec import _has_src1 as _spec_has_src1
import concourse.dve_ops as _dve_ops
from concourse.dve_uop import DveOpSpec as _DveOpSpec


def _register_dve_op(name, spec):
    if name in _dve_ops._SUB_OPCODE_FOR_NAME:
        return next(op for op in _dve_ops.OPS if op.name == name)
    opcode = _dve_ops._CUSTOM_DVE_ROW_BASE + len(_dve_ops.OPS)
    shas = {}
    for ver in ("v3", "v4"):
        try:
            so = _DveOpSpec(name=name, opcode=opcode,
                            uops=lower(spec, ver=ver),
                            rd1_en=_spec_has_src1(spec))
            shas[ver] = so.sha(ver)
        except Exception:
            pass
    op = _dve_ops.DveOp(name, spec, subdim=False, uops_sha=shas)
    _dve_ops.OPS.append(op)
    _dve_ops._SUB_OPCODE_FOR_NAME[name] = opcode
    _dve_ops.CUSTOM_DVE_SPECS[name] = spec
    return op


# nrm = clamp((t1 * isd) * gamma + beta, -imm2, imm2)
OP_NRM = _register_dve_op(
    "ANT_NRM_FUSED",
    Spec(body=minn(maxx((Src0 * Src1) * C0 + C1, Zero - C2), C2),
         reference=lambda in0, in1, s0, s1, imm2:
             np.minimum(np.maximum((in0 * in1) * s0 + s1, -imm2), imm2)))

# fused = (1 - 2r) * clip(nrm*s0 + s1, 0, 1)
OP_GATE = _register_dve_op(
    "ANT_TANH_GATE",
    Spec(body=((One - Src0) - Src0) * minn(maxx(Src1 * C0 + C1, Zero), One),
         reference=lambda in0, in1, s0, s1, imm2:
             (1.0 - 2.0 * in0) * np.clip(in1 * s0 + s1, 0.0, 1.0)))
AF = ActivationFunctionType

B, C, H, W = 16, 64, 256, 256
K = 3
G = 8
GS = C // G
EPS = 1e-05
NCORES = 8
BPC = B // NCORES          # samples per core

R = 4                      # output rows per block
WP = W + 4                 # padded input row width (2 left, 2 right)
WO = W + 2                 # conv output row width (x in [-1, W+1))
NBLK = H // R

MAX_N_F32 = 512            # fp32 moving-operand limit per matmul


def _build_nc():
    import os as _os
    nc = bacc.Bacc("TRN2", target_bir_lowering=False)
    x = nc.dram_tensor("x", [BPC, 128, H + 4, WP], BF16, kind="ExternalInput")
    wp = nc.dram_tensor("wp", [128, 3, C], BF16, kind="ExternalInput")
    ws = nc.dram_tensor("ws", [64, 3, C], BF16, kind="ExternalInput")
    wd = nc.dram_tensor("wd", [64, 9, C], BF16, kind="ExternalInput")
    wstat = nc.dram_tensor("wstat", [128, 128], BF16, kind="ExternalInput")
    ones = nc.dram_tensor("ones", [64, 1], BF16, kind="ExternalInput")
    cb = nc.dram_tensor("cb", [64, 1], F32, kind="ExternalInput")
    gam = nc.dram_tensor("gam", [64, 1], F32, kind="ExternalInput")
    bet = nc.dram_tensor("bet", [64, 1], F32, kind="ExternalInput")
    out = nc.dram_tensor("out", [BPC, H * W], F32, kind="ExternalOutput")

    NRI = R + 2            # input rows per block (img rows [y0-1, y0+R+1))
    NRO = R + 2            # xcv rows: y in [y0-1, y0+R+1)
    assert R % 2 == 0

    with TileContext(nc) as tc:
        with tc.tile_pool(name="consts", bufs=1) as consts, \
             tc.tile_pool(name="xin_p", bufs=3) as xin_p, \
             tc.tile_pool(name="xcv_p", bufs=3) as xcv_p, \
             tc.tile_pool(name="work", bufs=2) as work, \
             tc.tile_pool(name="outp", bufs=2) as outp, \
             tc.tile_pool(name="psA", bufs=2, space="PSUM") as psA, \
             tc.tile_pool(name="psB", bufs=2, space="PSUM") as psB:

            wpt = consts.tile([128, 3, C], BF16)
            wst = consts.tile([64, 3, C], BF16)
            wdt = consts.tile([64, 9, C], BF16)
            wstt = consts.tile([128, 128], BF16)
            onest = consts.tile([64, 1], BF16)
            cbt = consts.tile([64, 1], F32)
            gamt = consts.tile([64, 1], F32)
            bett = consts.tile([64, 1], F32)
            onesf = consts.tile([64, 1], F32)
            nc.vector.memset(onesf, 1.0)
            nc.sync.dma_start(out=wpt, in_=wp[:, :, :])
            nc.sync.dma_start(out=wst, in_=ws[:, :, :])
            nc.sync.dma_start(out=wdt, in_=wd[:, :, :])
            nc.sync.dma_start(out=wstt, in_=wstat[:, :])
            nc.sync.dma_start(out=onest, in_=ones[:, :])
            nc.sync.dma_start(out=cbt, in_=cb[:, :])
            nc.sync.dma_start(out=gamt, in_=gam[:, :])
            nc.sync.dma_start(out=bett, in_=bet[:, :])

            KREP = int(_os.environ.get("KREPEAT", "1"))
            KLOOP = int(_os.environ.get("KLOOP", "0"))

            def _body():
              for b in range(BPC):
                xcv_prev = None
                # iteration blk: conv block blk (if < NBLK), then full chain
                # for block blk-1 (whose xcv halo row R+1 is completed by
                # this iteration's combine).
                for blk in range(NBLK + 1):
                    if blk < NBLK:
                        y0 = blk * R
                        xin = xin_p.tile([128, NRI, WP], BF16, tag="xin")
                        nc.sync.dma_start(
                            out=xin, in_=x[b, :, y0 + 1:y0 + 1 + NRI, :])

                        # conv psum [64, R, W]: rows y in [y0, y0+R)
                        pc = psA.tile([64, R, W], F32, tag="psA")
                        for dy in range(3):
                            for j in range(0, R, 2):
                                rhs = bass.AP(
                                    tensor=xin.tensor,
                                    offset=xin.offset + (dy + j) * WP + 2,
                                    ap=[[xin.ap[0][0], 128], [WP, 2], [1, W]])
                                nc.tensor.matmul(pc[:, j:j + 2, :],
                                                 wpt[:, dy, :], rhs,
                                                 start=(dy == 0), stop=False)
                        for dy in range(3):
                            for j in range(0, R, 2):
                                rhs = bass.AP(
                                    tensor=xin.tensor,
                                    offset=xin.offset + (dy + j) * WP + 3,
                                    ap=[[xin.ap[0][0], 64], [WP, 2], [1, W]])
                                nc.tensor.matmul(pc[:, j:j + 2, :],
                                                 wst[:, dy, :], rhs,
                                                 start=False,
                                                 stop=(dy == 2 and j == R - 2))

                        # xcv ring tile [128, NRO, WO]: top x_conv / bottom sq
                        xcv = xcv_p.tile([128, NRO, WO], BF16, tag="xcv")
                        nc.gpsimd.memset(xcv[0:64, :, 0:1], 0.0)
                        nc.gpsimd.memset(xcv[0:64, :, WO - 1:WO], 0.0)
                        nc.scalar.activation(xcv[0:64, 1:R + 1, 1:WO - 1],
                                             pc, AF.Identity,
                                             bias=cbt, scale=1.0)
                        if blk == 0:
                            nc.gpsimd.memset(xcv[0:64, 0:1, 1:WO - 1], 0.0)
                        else:
                            # complete previous tile's top halo row and copy
                            # our bottom halo row from it
                            nc.scalar.activation(
                                xcv_prev[0:64, R + 1:R + 2, 1:WO - 1],
                                pc[:, 0:1, :], AF.Identity,
                                bias=cbt, scale=1.0)
                            nc.gpsimd.tensor_copy(
                                xcv[0:64, 0:1, :], xcv_prev[0:64, R:R + 1, :])
                    else:
                        # after last conv: previous tile's halo row is zero
                        nc.gpsimd.memset(
                            xcv_prev[0:64, R + 1:R + 2, 1:WO - 1], 0.0)

                    if blk >= 1:
                        cy0 = (blk - 1) * R
                        cxcv = xcv_prev
                        # ---- sq (bottom half) for stats: DVE bf16 mult
                        nc.gpsimd.tensor_tensor(
                            cxcv[64:128, 1:R + 1, 1:WO - 1],
                            cxcv[0:64, 1:R + 1, 1:WO - 1],
                            cxcv[0:64, 1:R + 1, 1:WO - 1], op=AluOpType.mult)

                        # ---- depthwise -> t1 = acc - mean_c + b (folded)
                        pd = psB.tile([64, R, W], F32, tag="psB")
                        t = 0
                        for dy in range(3):
                            for dx in range(3):
                                for j0r in range(0, R, 2):
                                    rhs = bass.AP(
                                        tensor=cxcv.tensor,
                                        offset=cxcv.offset + (dy + j0r) * WO + dx,
                                        ap=[[cxcv.ap[0][0], 64], [WO, 2], [1, W]])
                                    nc.tensor.matmul(
                                        pd[:, j0r:j0r + 2, :], wdt[:, t, :], rhs,
                                        start=(t == 0),
                                        stop=(t == 8 and j0r == R - 2))
                                t += 1
                        t1 = work.tile([64, R, W], F32, tag="t1")
                        nc.scalar.activation(t1, pd, AF.Identity,
                                             bias=cbt, scale=1.0)

                        # ---- stats psum [128, R, W]: mean_c / meansq_c
                        pstat = psA.tile([128, R, W], F32, tag="psA")
                        for j0r in range(0, R, 2):
                            rhs = bass.AP(
                                tensor=cxcv.tensor,
                                offset=cxcv.offset + (1 + j0r) * WO + 1,
                                ap=[[cxcv.ap[0][0], 128], [WO, 2], [1, W]])
                            nc.tensor.matmul(pstat[:, j0r:j0r + 2, :], wstt,
                                             rhs, start=True,
                                             stop=(j0r == R - 2))

                        # inv_std = exp(-0.5 ln(meansq - mean^2 + eps))
                        m2 = work.tile([64, R, W], F32, tag="m2")
                        nc.scalar.activation(m2, pstat[0:64], AF.Square)
                        veps = work.tile([64, R, W], F32, tag="veps")
                        nc.vector.scalar_tensor_tensor(
                            out=veps, in0=pstat[64:128], scalar=EPS, in1=m2,
                            op0=AluOpType.add, op1=AluOpType.subtract)
                        lnv = work.tile([64, R, W], F32, tag="lnv")
                        nc.scalar.activation(lnv, veps, AF.Ln)
                        isd = work.tile([64, R, W], F32, tag="isd")
                        nc.scalar.activation(isd, lnv, AF.Exp, scale=-0.5)

                        # ---- norm (fused): clamp((t1*isd)*g + b, +-30)
                        nrm = work.tile([64, R, W], F32, tag="nrm")
                        _fl = lambda a: a.rearrange("p a b -> p (a b)")
                        nc.vector._custom_dve(OP_NRM, out=_fl(nrm),
                                              in0=_fl(t1), in1=_fl(isd),
                                              s0=gamt, s1=bett, imm2=30.0)

                        # tanh(nrm) = 1 - 2/(1+exp(2nrm));
                        # r = exp(-ln(exp(2nrm) + 1))
                        ee = work.tile([64, R, W], F32, tag="ee")
                        nc.scalar.activation(ee, nrm, AF.Exp, scale=2.0)
                        lnd = work.tile([64, R, W], F32, tag="lnd")
                        nc.scalar.activation(lnd, ee, AF.Ln, bias=onesf)
                        rr = work.tile([64, R, W], F32, tag="rr")
                        nc.scalar.activation(rr, lnd, AF.Exp, scale=-1.0)
                        # fused = (1-2r)*clip(nrm/6+0.5,0,1)
                        zz = work.tile([64, R, W], F32, tag="zz")
                        nc.vector._custom_dve(OP_GATE, out=_fl(zz),
                                              in0=_fl(rr), in1=_fl(nrm),
                                              s0=1.0 / 6.0, s1=0.5)
                        xcv_int = bass.AP(
                            tensor=cxcv.tensor, offset=cxcv.offset + WO + 1,
                            ap=[[cxcv.ap[0][0], 64], [WO, R], [1, W]])
                        nc.gpsimd.tensor_tensor(zz, zz, xcv_int,
                                                 op=AluOpType.add)
                        ez = work.tile([64, R, W], BF16, tag="ez")
                        nc.scalar.activation(ez, zz, AF.Exp)

                        # ---- logsumexp: PE channel sum then ln
                        pl = psB.tile([1, R, W], F32, tag="psB")
                        for j0r in range(0, R, 2):
                            nc.tensor.matmul(
                                pl[:, j0r:j0r + 2, :], onest,
                                ez[:, j0r:j0r + 2, :],
                                start=True, stop=(j0r == R - 2))
                        lse = outp.tile([1, R, W], F32, tag="lse")
                        nc.scalar.activation(lse, pl, AF.Ln)
                        nc.sync.dma_start(
                            out=out[b, cy0 * W:(cy0 + R) * W].rearrange(
                                "(o a c) -> o a c", o=1, c=W),
                            in_=lse)

                    if blk < NBLK:
                        xcv_prev = xcv

            if KLOOP > 1:
                with tc.For_i(0, KLOOP, 1):
                    _body()
            else:
                for _rep in range(KREP):
                    _body()
    nc.compile()
    return nc


def _host_weights(conv_w, conv_b, gn_scale, gn_bias):
    w = np.asarray(conv_w, np.float32)
    wp = np.stack([np.concatenate([w[:, :, dy, 1].T, w[:, :, dy, 0].T], axis=0)
                   for dy in range(3)], axis=1).astype(np.float32)
    ws = np.stack([w[:, :, dy, 2].T for dy in range(3)], axis=1).astype(np.float32)

    wdiag = np.einsum('cckl->ckl', w)                       # [C, 3, 3]
    gsel = np.zeros((C, C), np.float32)
    for g in range(G):
        gsel[g * GS:(g + 1) * GS, g * GS:(g + 1) * GS] = 1.0 / GS
    dmats = []
    for dy in range(3):
        for dx in range(3):
            m = np.diag(wdiag[:, dy, dx]).astype(np.float32)
            if dy == 1 and dx == 1:
                m = m - gsel            # fold -mean_c (lhsT[ci,co]: G sym)
            dmats.append(m)
    wd = np.stack(dmats, axis=1).astype(np.float32)          # [64, 9, 64]

    wstat = np.zeros((128, 128), np.float32)
    wstat[0:64, 0:64] = gsel
    wstat[64:128, 64:128] = gsel

    ones = np.ones((64, 1), np.float32)
    cb = np.asarray(conv_b, np.float32).reshape(64, 1)
    gam = np.asarray(gn_scale, np.float32).reshape(64, 1)
    bet = np.asarray(gn_bias, np.float32).reshape(64, 1)
    import ml_dtypes
    bf = ml_dtypes.bfloat16
    return dict(wp=wp.astype(bf), ws=ws.astype(bf), wd=wd.astype(bf),
                wstat=wstat.astype(bf), ones=ones.astype(bf),
                cb=cb, gam=gam, bet=bet)


_NC_CACHE = None


def kernel(x, conv_w, conv_b, gn_scale, gn_bias):
    global _NC_CACHE
    x = np.asarray(x, np.float32)
    wts = _host_weights(conv_w, conv_b, gn_scale, gn_bias)
    if _NC_CACHE is None:
        _NC_CACHE = _build_nc()
    nc = _NC_CACHE
    import ml_dtypes
    xpad = np.zeros((B, 128, H + 4, WP), ml_dtypes.bfloat16)
    xpad[:, 0:64, 2:2 + H, 2:2 + W] = x
    xpad[:, 64:128, 2:2 + H, 3:3 + W] = x
    in_maps = []
    for c in range(NCORES):
        m = {"x": np.ascontiguousarray(xpad[c * BPC:(c + 1) * BPC])}
        m.update(wts)
        in_maps.append(m)
    import os as _os
    trace = bool(int(_os.environ.get("KTRACE", "0")))
    res = run_bass_kernel_spmd(nc, in_maps, core_ids=list(range(NCORES)),
                               trace=trace)
    kernel.exec_time_ns = res.exec_time_ns
    kernel.results_obj = res
    outs = [res.results[c]["out"].reshape(BPC, 1, H, W) for c in range(NCORES)]
    return np.concatenate(outs, axis=0)


if __name__ == "__main__":
    rng = np.random.default_rng(0)
    xs = rng.standard_normal((B, C, H, W), dtype=np.float32)
    wv = (rng.standard_normal((C, C, K, K), dtype=np.float32)
          / np.sqrt(C * K * K)).astype(np.float32)
    bv = (rng.standard_normal(C) * 0.05).astype(np.float32)
    gv = (1 + 0.05 * rng.standard_normal(C)).astype(np.float32)
    btv = (0.05 * rng.standard_normal(C)).astype(np.float32)
    o = kernel(xs, wv, bv, gv, btv)
    print(o.shape, o.dtype, float(o.mean()))

